# revision 48
# baseline (speedup 1.0000x reference)
"""Multi-head attention Trainium2 kernel (8 NeuronCores, tensor+data parallel).

Problem: B=2, S=2048, H=1024, NH=16 heads, DH=64, causal additive mask.
  qkv = hs @ w_qkv ; per-head scaled-dot-product attention ; out = ctx @ w_out

Sharding: core c owns batch b=c//4 and 4 heads g=(c%4)*4..+4.  Each core
computes QKV^T for its head slice, attention in transposed-score layout
(softmax along the PSUM partition axis, sums via a ones-row augmented V),
and a partial out-projection over its 256 head features; the host sums the
4 partials per batch.

All matmuls run as float32r (FP22 multiply, FP32 accumulate) which streams
at 1 cycle/row on the PE like bf16 but with 13 mantissa bits.

Schedule notes (v2):
- softmax normalization broadcasts 1/denom across partitions with a gpsimd
  partition_broadcast instead of a DRAM DMA round-trip (the round-trip
  blocked the in-order vector queue ~10us per (qc, pair)).
- attention kt loop is software-pipelined: scores+exp are emitted two k-tiles
  ahead of the E@V matmuls so the PE never waits on the scalar-engine exp.
- input DMAs are spread across 4 engine queues so the first qkv matmul isn't
  stuck behind 5MB of consts on one queue.
- the causal staircase multiply only covers the 128-wide diagonal block of
  each score window (columns beyond it are fully below the diagonal).
- out-proj PSUM->SBUF staging copies run on gpsimd to unload the vector queue.
"""

import sys

sys.path.insert(0, "/opt/trn_rl_repo")

import numpy as np

B, S, H, NH = 2, 2048, 1024, 16
DH = H // NH  # 64
N_CORES = 8
HEADS_PER_CORE = 4  # 2 pairs
SC = S // 512  # 4 q/s chunks of 512
KT = S // 128  # 16 k tiles of 128
F_CORE = HEADS_PER_CORE * DH  # 256 out-proj features per core

_CACHE = {}


def _build(mode):
    """Build + schedule the Bass program for `mode` in {"causal", "full"}."""
    import concourse.bass as bass
    import concourse.mybir as mybir
    from concourse import bacc
    from concourse.tile import TileContext

    f32 = mybir.dt.float32
    f32r = mybir.dt.float32r
    bf16 = mybir.dt.bfloat16
    fp16 = mybir.dt.float16
    EXP = mybir.ActivationFunctionType.Exp

    nc = bacc.Bacc("TRN2", target_bir_lowering=False, debug=False,
                   num_devices=N_CORES)

    hT = nc.dram_tensor("hT", [H, S], f32, kind="ExternalInput").ap()
    wqkv = nc.dram_tensor("wqkv", [H, 6 * 128], f32, kind="ExternalInput").ap()
    wo = nc.dram_tensor("wo", [F_CORE, H], f32, kind="ExternalInput").ap()
    strip = nc.dram_tensor("strip", [128, 2, 128], bf16, kind="ExternalInput").ap()
    ident_d = nc.dram_tensor("ident", [128, 128], f32, kind="ExternalInput").ap()
    ones_d = nc.dram_tensor("ones", [128, 64], bf16, kind="ExternalInput").ap()
    out = nc.dram_tensor("out", [S, H], f32, kind="ExternalOutput").ap()

    def n_kt(qc):  # k-tiles needed for q chunk qc
        return 4 * qc + 4 if mode == "causal" else KT

    with TileContext(nc) as tc:
        with (
            tc.tile_pool(name="consts", bufs=1) as consts,
            tc.tile_pool(name="persist", bufs=1) as persist,
            tc.tile_pool(name="stream", bufs=2) as stream,
            tc.tile_pool(name="epool", bufs=6) as epool,
            tc.tile_pool(name="norm", bufs=3) as norm,
            tc.tile_pool(name="outp", bufs=2) as outp,
            tc.tile_pool(name="psA", bufs=3, space="PSUM") as psA,
            tc.tile_pool(name="psB", bufs=1, space="PSUM") as psB,
        ):
            # ---- tiles ----------------------------------------------------
            wqkv_sb = consts.tile([128, 8, 6 * 128], f32, tag="wqkv")
            wo_sb = consts.tile([128, 2, H], f32, tag="wo")
            ident = consts.tile([128, 128], f32, tag="ident")
            strip_sb = None
            if mode == "causal":
                # staircase for the 128-wide diagonal block only
                strip_sb = consts.tile([128, 2, 128], bf16, tag="strip", name="strip_sb")
            # q,k blocks per s-chunk in fp16 (halves score-matmul weight loads
            # and SBUF reads; 10-bit mantissa keeps probs within ~0.6%):
            # [p, jt, 512]; jt: 0,1=q pairs, 2,3=k pairs
            qkT = [persist.tile([128, 4, 512], fp16, name=f"qkT{sc}", tag=f"qkT{sc}") for sc in range(SC)]
            # v blocks stay f32 for the PE transposes: [p, pair, 512]
            vT = [persist.tile([128, 2, 512], f32, name=f"vT{sc}", tag=f"vT{sc}") for sc in range(SC)]
            # ctx^T blocks per q-chunk: [p(f within pair), pair, 512]
            ctxT = [persist.tile([128, 2, 512], f32, name=f"ctxT{qc}", tag=f"ctxT{qc}") for qc in range(SC)]
            # v in normal layout per pair: cols 0:64=headA, 64=ones, 65:129=headB, 129=ones
            v2 = persist.tile([128, 2, KT, 130], bf16, tag="v2")

            def emit_const_loads():
                # gpsimd queue: wqkv in 2-ht row groups, q+k columns first
                # (first needed by the jt loop and attention scores), v after
                wqkv_r = wqkv.rearrange("(ht p) j -> p ht j", p=128)
                for hg in range(4):
                    nc.gpsimd.dma_start(
                        out=wqkv_sb[:, 2 * hg:2 * hg + 2, 0:512].bitcast(f32r),
                        in_=wqkv_r[:, 2 * hg:2 * hg + 2, 0:512].bitcast(f32r))
                for hg in range(4):
                    nc.gpsimd.dma_start(
                        out=wqkv_sb[:, 2 * hg:2 * hg + 2, 512:768].bitcast(f32r),
                        in_=wqkv_r[:, 2 * hg:2 * hg + 2, 512:768].bitcast(f32r))
                # scalar queue: small consts needed early, then wo (needed last)
                nc.scalar.dma_start(out=ident[:].bitcast(f32r), in_=ident_d.bitcast(f32r))
                if strip_sb is not None:
                    nc.scalar.dma_start(out=strip_sb, in_=strip)
                nc.scalar.dma_start(
                    out=bass.AP(tensor=v2.tensor, offset=v2[:].offset + 64,
                                ap=[[v2[:].ap[0][0], 128], [65, 2 * KT * 2], [1, 1]]),
                    in_=bass.AP(tensor=ones_d.tensor, offset=0,
                                ap=[[64, 128], [0, 2 * KT * 2], [1, 1]]),
                )
                nc.scalar.dma_start(out=wo_sb.bitcast(f32r),
                                    in_=wo.rearrange("(ft p) e -> p ft e", p=128).bitcast(f32r))

            def emit_outproj(blk):
                """Final-block out-projection: staging copies alternate
                between scalar and vector, and each row-block's write is
                split across two DMA queues to halve the tail drain."""
                for i in range(4):
                    st = blk * 4 + i
                    po = psA.tile([128, 1024], f32, tag="psA", name="po")
                    for ec in range(2):
                        for ft in range(2):
                            nc.tensor.matmul(
                                po[:, ec * 512:(ec + 1) * 512],
                                lhsT=ctxT[blk][:, ft, i * 128:(i + 1) * 128].bitcast(f32r),
                                rhs=wo_sb[:, ft, ec * 512:(ec + 1) * 512].bitcast(f32r),
                                start=(ft == 0), stop=(ft == 1),
                            )
                    o_sb = outp.tile([128, 1024], f32, tag="o_sb", name="o_sb")
                    if i % 2 == 0:
                        nc.scalar.copy(o_sb, po)
                    else:
                        nc.vector.tensor_copy(o_sb, po)
                    nc.sync.dma_start(out=out[st * 128:st * 128 + 64, :], in_=o_sb[0:64, :])
                    nc.gpsimd.dma_start(out=out[st * 128 + 64:(st + 1) * 128, :], in_=o_sb[64:128, :])

            def emit_qkv(sc, qk_only=False):
                # ==== QKV^T for s-chunk sc =================================
                hT_t = stream.tile([128, 8, 512], f32, tag="hT", name="hT_t")
                hT_r = hT.rearrange("(ht p) s -> p ht s", p=128)[:, :, sc * 512:(sc + 1) * 512]
                if sc == 0:
                    # per-ht splits across two queues so the first qkv matmul
                    # only waits for the first 256KB, not the full 2MB
                    for ht in range(4):
                        nc.sync.dma_start(out=hT_t[:, ht, :].bitcast(f32r), in_=hT_r[:, ht, :].bitcast(f32r))
                        nc.scalar.dma_start(out=hT_t[:, 4 + ht, :].bitcast(f32r), in_=hT_r[:, 4 + ht, :].bitcast(f32r))
                    emit_const_loads()
                else:
                    nc.sync.dma_start(out=hT_t.bitcast(f32r), in_=hT_r.bitcast(f32r))
                def emit_jt(jt):
                    ps = psA.tile([128, 1024], f32, tag="psA", name="ps")
                    acc = ps[:, 0:512]
                    for ht in range(8):
                        nc.tensor.matmul(
                            acc,
                            lhsT=wqkv_sb[:, ht, jt * 128:(jt + 1) * 128].bitcast(f32r),
                            rhs=hT_t[:, ht, :].bitcast(f32r),
                            start=(ht == 0),
                            stop=(ht == 7),
                        )
                    if jt < 4:
                        nc.vector.tensor_copy(qkT[sc][:, jt, :], acc)
                    else:
                        nc.vector.tensor_copy(vT[sc][:, jt - 4, :].bitcast(f32r), acc)

                # q,k first (scores), then each v block followed directly by
                # its transposes so v2 is ready when the ctx matmuls start
                for jt in range(4):
                    emit_jt(jt)
                if qk_only:
                    # v projections + transposes ride as attention(0) fillers
                    from functools import partial
                    return [partial(emit_jt, 4), partial(emit_transposes, sc, 0),
                            partial(emit_jt, 5), partial(emit_transposes, sc, 1)]
                emit_jt(4)
                emit_transposes(sc, 0)
                emit_jt(5)
                emit_transposes(sc, 1)
                return []

            def emit_transposes(sc, pair):
                # ==== v transposes for the 4 new k-tiles of one pair ========
                for i in range(4):
                    kt = sc * 4 + i
                    pst = psA.tile([128, 1024], f32, tag="psA", name="pst")
                    nc.tensor.transpose(
                        pst[:, 0:128].bitcast(f32r),
                        in_=vT[sc][:, pair, i * 128:(i + 1) * 128].bitcast(f32r),
                        identity=ident[:].bitcast(f32r),
                    )
                    # one copy per transpose (f32 PSUM -> bf16 SBUF):
                    # headA -> cols 0:64, headB -> 65:129
                    dst = v2[:, pair, kt, :]
                    nc.vector.tensor_copy(
                        bass.AP(tensor=v2.tensor, offset=dst.offset,
                                ap=[dst.ap[0], [65, 2], [1, 64]]),
                        pst[:, 0:128].rearrange("p (h d) -> p h d", h=2))

            def make_qkv_fillers(sc):
                """DMA starter + PE-filler closures for chunk sc's QKV.

                Each filler is ~1us of PE work (half a jt accumulation or a
                pair of V transposes) inserted into the exp-limited attention
                loop of the previous chunk, where the PE would otherwise idle.
                """
                st = {}

                def dma():
                    hT_t = stream.tile([128, 8, 512], f32, tag="hT", name="hT_t")
                    st["h"] = hT_t
                    hT_r = hT.rearrange("(ht p) s -> p ht s", p=128)[:, :, sc * 512:(sc + 1) * 512]
                    nc.sync.dma_start(out=hT_t.bitcast(f32r), in_=hT_r.bitcast(f32r))

                def jt_half(jt, half):
                    if half == 0:
                        st[jt] = psA.tile([128, 1024], f32, tag="psA", name="ps")
                    acc = st[jt][:, 0:512]
                    for ht in range(4 * half, 4 * half + 4):
                        nc.tensor.matmul(
                            acc,
                            lhsT=wqkv_sb[:, ht, jt * 128:(jt + 1) * 128].bitcast(f32r),
                            rhs=st["h"][:, ht, :].bitcast(f32r),
                            start=(ht == 0),
                            stop=(ht == 7),
                        )
                    if half == 1:
                        if jt < 4:
                            nc.vector.tensor_copy(qkT[sc][:, jt, :], acc)
                        else:
                            nc.vector.tensor_copy(vT[sc][:, jt - 4, :].bitcast(f32r), acc)
                        del st[jt]

                from functools import partial
                fillers = []
                for jt in (0, 1, 2, 3):
                    fillers.append(partial(jt_half, jt, 0))
                    fillers.append(partial(jt_half, jt, 1))
                fillers.append(partial(jt_half, 4, 0))
                fillers.append(partial(jt_half, 4, 1))
                fillers.append(partial(emit_transposes, sc, 0))
                fillers.append(partial(jt_half, 5, 0))
                fillers.append(partial(jt_half, 5, 1))
                fillers.append(partial(emit_transposes, sc, 1))
                return dma, fillers

            def make_outproj_fillers(blk):
                """Per-s-tile out-projection fillers (vector staging copy)."""
                def po_tile(i):
                    st_ = blk * 4 + i
                    po = psA.tile([128, 1024], f32, tag="psA", name="po")
                    for ec in range(2):
                        for ft in range(2):
                            nc.tensor.matmul(
                                po[:, ec * 512:(ec + 1) * 512],
                                lhsT=ctxT[blk][:, ft, i * 128:(i + 1) * 128].bitcast(f32r),
                                rhs=wo_sb[:, ft, ec * 512:(ec + 1) * 512].bitcast(f32r),
                                start=(ft == 0), stop=(ft == 1),
                            )
                    o_sb = outp.tile([128, 1024], f32, tag="o_sb", name="o_sb")
                    nc.vector.tensor_copy(o_sb, po)
                    nc.sync.dma_start(out=out[st_ * 128:(st_ + 1) * 128, :], in_=o_sb)

                from functools import partial
                return [partial(po_tile, i) for i in range(4)]

            def emit_attention(qc, fillers=()):
                fillers = list(fillers)
                nkt = n_kt(qc)
                for pair in range(2):
                    # headA ctx in cols 0:512, headB in 512:1024 of one tile
                    ctx2 = psB.tile([65, 1024], f32, tag="ctx2", name="ctx2")
                    E_tiles = [None] * nkt

                    def emit_scores_exp(kt):
                        # diagonal tiles only need columns j >= 128*t
                        diag = mode == "causal" and kt >= 4 * qc
                        w0 = 128 * (kt - 4 * qc) if diag else 0
                        sp = psA.tile([128, 1024], f32, tag="psA")
                        kblk, ki = qkT[kt // 4], (kt % 4) * 128
                        # transposed scores, 2 heads row-packed on the PE
                        nc.tensor.matmul(
                            sp[:, w0:512],
                            lhsT=kblk[0:64, 2 + pair, ki:ki + 128],
                            rhs=qkT[qc][0:64, 0 + pair, w0:512],
                            start=True, stop=True,
                        )
                        nc.tensor.matmul(
                            sp[:, 512 + w0:1024],
                            lhsT=kblk[64:128, 2 + pair, ki:ki + 128],
                            rhs=qkT[qc][64:128, 0 + pair, w0:512],
                            start=True, stop=True,
                        )
                        E = epool.tile([128, 2, 512], bf16, tag="E")
                        nc.scalar.activation(
                            E[:, :, w0:512],
                            sp[:].rearrange("p (two q) -> p two q", two=2)[:, :, w0:512],
                            EXP)
                        if diag:
                            # zero the strictly-masked staircase in the 128-wide
                            # diagonal block; columns beyond it are fully valid
                            nc.vector.tensor_mul(
                                E[:, :, w0:w0 + 128],
                                E[:, :, w0:w0 + 128],
                                strip_sb)
                        E_tiles[kt] = (E, w0)

                    # software pipeline: scores+exp run 2 k-tiles ahead of E@V,
                    # and ctx matmuls are grouped head-major per kt-pair so
                    # consecutive accumulates hit the same PSUM region
                    emit_scores_exp(0)
                    if fillers:
                        fillers.pop(0)()
                    emit_scores_exp(1)
                    if fillers:
                        fillers.pop(0)()
                    for kt0 in range(0, nkt, 2):
                        for k in (kt0 + 2, kt0 + 3):
                            if k < nkt:
                                emit_scores_exp(k)
                        for head in range(2):
                            for k in (kt0, kt0 + 1):
                                E, w0 = E_tiles[k]
                                nc.tensor.matmul(
                                    ctx2[:, head * 512 + w0:head * 512 + 512],
                                    lhsT=v2[:, pair, k, head * 65:head * 65 + 65],
                                    rhs=E[:, head, w0:512],
                                    start=(k == 0), stop=(k == nkt - 1),
                                )
                        E_tiles[kt0] = E_tiles[kt0 + 1] = None
                        # fill the PE's exp-wait slack with next-chunk qkv /
                        # previous-chunk outproj work
                        if fillers:
                            fillers.pop(0)()
                    # normalization: rows 64 hold the softmax denominators.
                    # copy to SBUF (recip reads partition 0 only), reciprocal,
                    # then a 64-partition gpsimd broadcast (both multiplies
                    # read base-partition-0 slices) — no DRAM round-trip.
                    r_sb = norm.tile([1, 1024], f32, tag="r_sb", name="r_sb")
                    nc.vector.tensor_copy(r_sb[0:1, :], ctx2[64:65, :])
                    rr = norm.tile([1, 1024], f32, tag="rr", name="rr")
                    nc.vector.reciprocal_approx_fast(out=rr[:], in_=r_sb[:])
                    rbc = norm.tile([64, 1024], f32, tag="rbc", name="rbc")
                    nc.gpsimd.partition_broadcast(rbc[:, :], rr[0:1, :])
                    nc.vector.tensor_mul(ctxT[qc][0:64, pair, :].bitcast(f32r), ctx2[0:64, 0:512], rbc[0:64, 0:512])
                    nc.vector.tensor_mul(ctxT[qc][64:128, pair, :].bitcast(f32r), ctx2[0:64, 512:1024], rbc[0:64, 512:1024])
                return fillers

            if mode == "causal":
                # attention(qc) only needs k-chunks <= qc. The next chunk's
                # QKV and the previous chunk's out-projection run as PE
                # fillers inside the attention loop.
                carry = emit_qkv(0, qk_only=True)
                for sc in range(SC):
                    fillers = list(carry)
                    carry = []
                    if sc + 1 < SC:
                        dma, qf = make_qkv_fillers(sc + 1)
                        dma()
                        fillers += qf
                    if sc > 0:
                        fillers += make_outproj_fillers(sc - 1)
                    rest = emit_attention(sc, fillers)
                    for f in rest:
                        f()
            else:
                # full attention needs every k-chunk before any q-chunk
                for sc in range(SC):
                    emit_qkv(sc)
                for qc in range(SC):
                    emit_attention(qc)
                    if qc > 0:
                        emit_outproj(qc - 1)
            emit_outproj(SC - 1)

    nc.compile()
    return nc


def _get_program(mode):
    if mode not in _CACHE:
        _CACHE[mode] = _build(mode)
    return _CACHE[mode]


def _classify_mask(mask):
    """Return "causal", "full", or "generic"."""
    m = mask.reshape(B, S, S)
    tril = np.tril_indices(S)
    if np.all(m == 0.0):
        return "full"
    for b in range(B):
        mb = m[b]
        if not np.all(mb[tril] == 0.0):
            return "generic"
        if not np.all(mb[np.triu_indices(S, k=1)] < -240.0):
            return "generic"
    return "causal"


def _prepare_in_maps(hidden_states, w_qkv, w_out):
    from ml_dtypes import bfloat16

    # strip[i, d, j] = 1.0 iff j >= i — the 128-wide diagonal staircase
    base = (np.arange(128, dtype=np.int32)[None, :] >= np.arange(128, dtype=np.int32)[:, None]).astype(bfloat16)
    strip = np.ascontiguousarray(np.broadcast_to(base[:, None, :], (128, 2, 128)))

    hT = [np.ascontiguousarray(hidden_states[b].T) for b in range(B)]

    in_maps = []
    for c in range(N_CORES):
        b, g = divmod(c, 4)
        cols = []
        for part in (0, 1, 2):  # q, k, v column groups of w_qkv
            for pair in range(2):
                for h in (4 * g + 2 * pair, 4 * g + 2 * pair + 1):
                    cols.append(w_qkv[:, part * H + h * DH: part * H + (h + 1) * DH])
        # order above is [q(pair0 h0,h1), q(pair1 h2,h3), k..., v...] = jt 0..5
        wqkv_c = np.concatenate(cols, axis=1)
        # fold the 1/sqrt(DH) score scale into the q columns
        wqkv_c = np.ascontiguousarray(wqkv_c)
        wqkv_c[:, 0:256] *= 1.0 / np.sqrt(DH)
        wo_c = np.ascontiguousarray(w_out[g * F_CORE:(g + 1) * F_CORE, :])
        in_maps.append({"hT": hT[b], "wqkv": wqkv_c, "wo": wo_c, "strip": strip,
                        "ident": np.eye(128, dtype=np.float32),
                        "ones": np.ones((128, 64), dtype=bfloat16)})
    return in_maps


def _run(inputs, trace=False):
    from concourse.bass_utils import run_bass_kernel_spmd

    hidden_states = np.asarray(inputs["hidden_states"], dtype=np.float32)
    mask = np.asarray(inputs["attention_mask"], dtype=np.float32)
    w_qkv = np.asarray(inputs["w_qkv"], dtype=np.float32)
    w_out = np.asarray(inputs["w_out"], dtype=np.float32)

    mode = _classify_mask(mask)
    if mode == "generic":
        return _numpy_reference(hidden_states, mask, w_qkv, w_out), None

    nc = _get_program(mode)
    in_maps = _prepare_in_maps(hidden_states, w_qkv, w_out)
    res = run_bass_kernel_spmd(nc, in_maps, list(range(N_CORES)), trace=trace)
    out = np.zeros((B, S, H), dtype=np.float32)
    for c in range(N_CORES):
        out[c // 4] += res.results[c]["out"]
    return out, res


def kernel(**inputs):
    out, _ = _run(inputs, trace=False)
    return out


def kernel_traced(**inputs):
    """Like kernel() but with NTFF profiling; returns (out, BassKernelResults)."""
    return _run(inputs, trace=True)


def _numpy_reference(hidden_states, mask, w_qkv, w_out):
    """Exact fallback for unrecognized masks (slow, chunked numpy)."""
    out = np.zeros((B, S, H), dtype=np.float32)
    m = mask.reshape(B, 1, S, S)
    for b in range(B):
        qkv = hidden_states[b] @ w_qkv  # [S, 3H]
        q = qkv[:, 0:H].reshape(S, NH, DH)
        k = qkv[:, H:2 * H].reshape(S, NH, DH)
        v = qkv[:, 2 * H:].reshape(S, NH, DH)
        ctx = np.zeros((S, NH, DH), dtype=np.float32)
        for h in range(NH):
            s = (q[:, h] @ k[:, h].T) / np.sqrt(DH) + m[b, 0]
            s = s - s.max(axis=-1, keepdims=True)
            e = np.exp(s)
            p = e / e.sum(axis=-1, keepdims=True)
            ctx[:, h] = p @ v[:, h]
        out[b] = ctx.reshape(S, H) @ w_out
    return out


# revision 49
# speedup vs baseline: 1.1716x; 1.1716x over previous
"""Multi-head attention Trainium2 kernel (8 NeuronCores, tensor+data parallel).

Problem: B=2, S=2048, H=1024, NH=16 heads, DH=64, causal additive mask.
  qkv = hs @ w_qkv ; per-head scaled-dot-product attention ; out = ctx @ w_out

Sharding: core c owns batch b=c//4 and 4 heads g=(c%4)*4..+4.  Each core
computes QKV^T for its head slice, attention in transposed-score layout
(softmax along the PSUM partition axis, sums via a ones-row augmented V),
and a partial out-projection over its 256 head features; the host sums the
4 partials per batch.

All matmuls run as float32r (FP22 multiply, FP32 accumulate) which streams
at 1 cycle/row on the PE like bf16 but with 13 mantissa bits.

Schedule notes (v2):
- softmax normalization broadcasts 1/denom across partitions with a gpsimd
  partition_broadcast instead of a DRAM DMA round-trip (the round-trip
  blocked the in-order vector queue ~10us per (qc, pair)).
- attention kt loop is software-pipelined: scores+exp are emitted two k-tiles
  ahead of the E@V matmuls so the PE never waits on the scalar-engine exp.
- input DMAs are spread across 4 engine queues so the first qkv matmul isn't
  stuck behind 5MB of consts on one queue.
- the causal staircase multiply only covers the 128-wide diagonal block of
  each score window (columns beyond it are fully below the diagonal).
- out-proj PSUM->SBUF staging copies run on gpsimd to unload the vector queue.
"""

import sys

sys.path.insert(0, "/opt/trn_rl_repo")

import numpy as np

B, S, H, NH = 2, 2048, 1024, 16
DH = H // NH  # 64
N_CORES = 8
HEADS_PER_CORE = 4  # 2 pairs
SC = S // 512  # 4 q/s chunks of 512
KT = S // 128  # 16 k tiles of 128
F_CORE = HEADS_PER_CORE * DH  # 256 out-proj features per core

_CACHE = {}


def _build(mode):
    """Build + schedule the Bass program for `mode` in {"causal", "full"}."""
    import concourse.bass as bass
    import concourse.mybir as mybir
    from concourse import bacc
    from concourse.tile import TileContext

    f32 = mybir.dt.float32
    f32r = mybir.dt.float32r
    bf16 = mybir.dt.bfloat16
    fp16 = mybir.dt.float16
    EXP = mybir.ActivationFunctionType.Exp

    nc = bacc.Bacc("TRN2", target_bir_lowering=False, debug=False,
                   num_devices=N_CORES)

    hT = nc.dram_tensor("hT", [H, S], f32, kind="ExternalInput").ap()
    wqkv = nc.dram_tensor("wqkv", [H, 6 * 128], f32, kind="ExternalInput").ap()
    wo = nc.dram_tensor("wo", [F_CORE, H], f32, kind="ExternalInput").ap()
    strip = nc.dram_tensor("strip", [128, 2, 128], bf16, kind="ExternalInput").ap()
    ident_d = nc.dram_tensor("ident", [128, 128], f32, kind="ExternalInput").ap()
    ones_d = nc.dram_tensor("ones", [128, 64], bf16, kind="ExternalInput").ap()
    out = nc.dram_tensor("out", [S, H], f32, kind="ExternalOutput").ap()

    def n_kt(qc):  # k-tiles needed for q chunk qc
        return 4 * qc + 4 if mode == "causal" else KT

    with TileContext(nc) as tc:
        with (
            tc.tile_pool(name="consts", bufs=1) as consts,
            tc.tile_pool(name="persist", bufs=1) as persist,
            tc.tile_pool(name="stream", bufs=2) as stream,
            tc.tile_pool(name="epool", bufs=6) as epool,
            tc.tile_pool(name="norm", bufs=3) as norm,
            tc.tile_pool(name="outp", bufs=2) as outp,
            tc.tile_pool(name="psA", bufs=3, space="PSUM") as psA,
            tc.tile_pool(name="psB", bufs=1, space="PSUM") as psB,
        ):
            # ---- tiles ----------------------------------------------------
            wqkv_sb = consts.tile([128, 8, 6 * 128], f32, tag="wqkv")
            wo_sb = consts.tile([128, 2, H], f32, tag="wo")
            ident = consts.tile([128, 128], f32, tag="ident")
            strip_sb = None
            if mode == "causal":
                # staircase for the 128-wide diagonal block only
                strip_sb = consts.tile([128, 2, 128], bf16, tag="strip", name="strip_sb")
            # q,k blocks per s-chunk in fp16 (halves score-matmul weight loads
            # and SBUF reads; 10-bit mantissa keeps probs within ~0.6%):
            # [p, jt, 512]; jt: 0,1=q pairs, 2,3=k pairs
            qkT = [persist.tile([128, 4, 512], fp16, name=f"qkT{sc}", tag=f"qkT{sc}") for sc in range(SC)]
            # v blocks stay f32 for the PE transposes: [p, pair, 512]
            vT = [persist.tile([128, 2, 512], f32, name=f"vT{sc}", tag=f"vT{sc}") for sc in range(SC)]
            # ctx^T blocks per q-chunk: [p(f within pair), pair, 512]
            ctxT = [persist.tile([128, 2, 512], f32, name=f"ctxT{qc}", tag=f"ctxT{qc}") for qc in range(SC)]
            # v in normal layout per pair: cols 0:64=headA, 64=ones, 65:129=headB, 129=ones
            v2 = persist.tile([128, 2, KT, 130], bf16, tag="v2")

            def emit_const_loads():
                # gpsimd queue: wqkv in 2-ht row groups, q+k columns first
                # (first needed by the jt loop and attention scores), v after
                wqkv_r = wqkv.rearrange("(ht p) j -> p ht j", p=128)
                for hg in range(4):
                    nc.gpsimd.dma_start(
                        out=wqkv_sb[:, 2 * hg:2 * hg + 2, 0:512].bitcast(f32r),
                        in_=wqkv_r[:, 2 * hg:2 * hg + 2, 0:512].bitcast(f32r))
                for hg in range(4):
                    nc.gpsimd.dma_start(
                        out=wqkv_sb[:, 2 * hg:2 * hg + 2, 512:768].bitcast(f32r),
                        in_=wqkv_r[:, 2 * hg:2 * hg + 2, 512:768].bitcast(f32r))
                # scalar queue: small consts needed early, then wo (needed last)
                nc.scalar.dma_start(out=ident[:].bitcast(f32r), in_=ident_d.bitcast(f32r))
                if strip_sb is not None:
                    nc.scalar.dma_start(out=strip_sb, in_=strip)
                nc.scalar.dma_start(
                    out=bass.AP(tensor=v2.tensor, offset=v2[:].offset + 64,
                                ap=[[v2[:].ap[0][0], 128], [65, 2 * KT * 2], [1, 1]]),
                    in_=bass.AP(tensor=ones_d.tensor, offset=0,
                                ap=[[64, 128], [0, 2 * KT * 2], [1, 1]]),
                )
                nc.scalar.dma_start(out=wo_sb.bitcast(f32r),
                                    in_=wo.rearrange("(ft p) e -> p ft e", p=128).bitcast(f32r))

            def emit_outproj(blk):
                """Final-block out-projection: staging copies alternate
                between scalar and vector, and each row-block's write is
                split across two DMA queues to halve the tail drain."""
                for i in range(4):
                    st = blk * 4 + i
                    po = psA.tile([128, 1024], f32, tag="psA", name="po")
                    for ec in range(2):
                        for ft in range(2):
                            nc.tensor.matmul(
                                po[:, ec * 512:(ec + 1) * 512],
                                lhsT=ctxT[blk][:, ft, i * 128:(i + 1) * 128].bitcast(f32r),
                                rhs=wo_sb[:, ft, ec * 512:(ec + 1) * 512].bitcast(f32r),
                                start=(ft == 0), stop=(ft == 1),
                            )
                    o_sb = outp.tile([128, 1024], f32, tag="o_sb", name="o_sb")
                    if i % 2 == 0:
                        nc.scalar.copy(o_sb, po)
                    else:
                        nc.vector.tensor_copy(o_sb, po)
                    nc.sync.dma_start(out=out[st * 128:(st + 1) * 128, :], in_=o_sb)

            def emit_qkv(sc, qk_only=False):
                # ==== QKV^T for s-chunk sc =================================
                hT_t = stream.tile([128, 8, 512], f32, tag="hT", name="hT_t")
                hT_r = hT.rearrange("(ht p) s -> p ht s", p=128)[:, :, sc * 512:(sc + 1) * 512]
                if sc == 0:
                    # per-ht splits across two queues so the first qkv matmul
                    # only waits for the first 256KB, not the full 2MB
                    for ht in range(4):
                        nc.sync.dma_start(out=hT_t[:, ht, :].bitcast(f32r), in_=hT_r[:, ht, :].bitcast(f32r))
                        nc.scalar.dma_start(out=hT_t[:, 4 + ht, :].bitcast(f32r), in_=hT_r[:, 4 + ht, :].bitcast(f32r))
                    emit_const_loads()
                else:
                    nc.sync.dma_start(out=hT_t.bitcast(f32r), in_=hT_r.bitcast(f32r))
                def emit_jt(jt):
                    ps = psA.tile([128, 1024], f32, tag="psA", name="ps")
                    acc = ps[:, 0:512]
                    for ht in range(8):
                        nc.tensor.matmul(
                            acc,
                            lhsT=wqkv_sb[:, ht, jt * 128:(jt + 1) * 128].bitcast(f32r),
                            rhs=hT_t[:, ht, :].bitcast(f32r),
                            start=(ht == 0),
                            stop=(ht == 7),
                        )
                    if jt < 4:
                        nc.vector.tensor_copy(qkT[sc][:, jt, :], acc)
                    else:
                        nc.vector.tensor_copy(vT[sc][:, jt - 4, :].bitcast(f32r), acc)

                # q,k first (scores), then each v block followed directly by
                # its transposes so v2 is ready when the ctx matmuls start
                for jt in range(4):
                    emit_jt(jt)
                if qk_only:
                    # v projections + transposes ride as attention(0) fillers
                    from functools import partial
                    return [partial(emit_jt, 4), partial(emit_transposes, sc, 0),
                            partial(emit_jt, 5), partial(emit_transposes, sc, 1)]
                emit_jt(4)
                emit_transposes(sc, 0)
                emit_jt(5)
                emit_transposes(sc, 1)
                return []

            def emit_transposes(sc, pair):
                # ==== v transposes for the 4 new k-tiles of one pair ========
                for i in range(4):
                    kt = sc * 4 + i
                    pst = psA.tile([128, 1024], f32, tag="psA", name="pst")
                    nc.tensor.transpose(
                        pst[:, 0:128].bitcast(f32r),
                        in_=vT[sc][:, pair, i * 128:(i + 1) * 128].bitcast(f32r),
                        identity=ident[:].bitcast(f32r),
                    )
                    # one copy per transpose (f32 PSUM -> bf16 SBUF):
                    # headA -> cols 0:64, headB -> 65:129
                    dst = v2[:, pair, kt, :]
                    nc.vector.tensor_copy(
                        bass.AP(tensor=v2.tensor, offset=dst.offset,
                                ap=[dst.ap[0], [65, 2], [1, 64]]),
                        pst[:, 0:128].rearrange("p (h d) -> p h d", h=2))

            def make_qkv_fillers(sc):
                """DMA starter + PE-filler closures for chunk sc's QKV.

                Each filler is ~1us of PE work (half a jt accumulation or a
                pair of V transposes) inserted into the exp-limited attention
                loop of the previous chunk, where the PE would otherwise idle.
                """
                st = {}

                def dma():
                    hT_t = stream.tile([128, 8, 512], f32, tag="hT", name="hT_t")
                    st["h"] = hT_t
                    hT_r = hT.rearrange("(ht p) s -> p ht s", p=128)[:, :, sc * 512:(sc + 1) * 512]
                    nc.sync.dma_start(out=hT_t.bitcast(f32r), in_=hT_r.bitcast(f32r))

                def jt_half(jt, half):
                    if half == 0:
                        st[jt] = psA.tile([128, 1024], f32, tag="psA", name="ps")
                    acc = st[jt][:, 0:512]
                    for ht in range(4 * half, 4 * half + 4):
                        nc.tensor.matmul(
                            acc,
                            lhsT=wqkv_sb[:, ht, jt * 128:(jt + 1) * 128].bitcast(f32r),
                            rhs=st["h"][:, ht, :].bitcast(f32r),
                            start=(ht == 0),
                            stop=(ht == 7),
                        )
                    if half == 1:
                        if jt < 4:
                            nc.vector.tensor_copy(qkT[sc][:, jt, :], acc)
                        else:
                            nc.vector.tensor_copy(vT[sc][:, jt - 4, :].bitcast(f32r), acc)
                        del st[jt]

                from functools import partial
                fillers = []
                for jt in (0, 1, 2, 3):
                    fillers.append(partial(jt_half, jt, 0))
                    fillers.append(partial(jt_half, jt, 1))
                fillers.append(partial(jt_half, 4, 0))
                fillers.append(partial(jt_half, 4, 1))
                fillers.append(partial(emit_transposes, sc, 0))
                fillers.append(partial(jt_half, 5, 0))
                fillers.append(partial(jt_half, 5, 1))
                fillers.append(partial(emit_transposes, sc, 1))
                return dma, fillers

            def make_outproj_fillers(blk):
                """Per-s-tile out-projection fillers (vector staging copy)."""
                def po_tile(i):
                    st_ = blk * 4 + i
                    po = psA.tile([128, 1024], f32, tag="psA", name="po")
                    for ec in range(2):
                        for ft in range(2):
                            nc.tensor.matmul(
                                po[:, ec * 512:(ec + 1) * 512],
                                lhsT=ctxT[blk][:, ft, i * 128:(i + 1) * 128].bitcast(f32r),
                                rhs=wo_sb[:, ft, ec * 512:(ec + 1) * 512].bitcast(f32r),
                                start=(ft == 0), stop=(ft == 1),
                            )
                    o_sb = outp.tile([128, 1024], f32, tag="o_sb", name="o_sb")
                    nc.vector.tensor_copy(o_sb, po)
                    nc.sync.dma_start(out=out[st_ * 128:(st_ + 1) * 128, :], in_=o_sb)

                from functools import partial
                return [partial(po_tile, i) for i in range(4)]

            def emit_attention(qc, fillers=()):
                fillers = list(fillers)
                nkt = n_kt(qc)
                for pair in range(2):
                    # headA ctx in cols 0:512, headB in 512:1024 of one tile
                    ctx2 = psB.tile([65, 1024], f32, tag="ctx2", name="ctx2")
                    E_tiles = [None] * nkt

                    def emit_scores_exp(kt):
                        # diagonal tiles only need columns j >= 128*t
                        diag = mode == "causal" and kt >= 4 * qc
                        w0 = 128 * (kt - 4 * qc) if diag else 0
                        sp = psA.tile([128, 1024], f32, tag="psA")
                        kblk, ki = qkT[kt // 4], (kt % 4) * 128
                        # transposed scores, 2 heads row-packed on the PE
                        nc.tensor.matmul(
                            sp[:, w0:512],
                            lhsT=kblk[0:64, 2 + pair, ki:ki + 128],
                            rhs=qkT[qc][0:64, 0 + pair, w0:512],
                            start=True, stop=True,
                        )
                        nc.tensor.matmul(
                            sp[:, 512 + w0:1024],
                            lhsT=kblk[64:128, 2 + pair, ki:ki + 128],
                            rhs=qkT[qc][64:128, 0 + pair, w0:512],
                            start=True, stop=True,
                        )
                        E = epool.tile([128, 2, 512], bf16, tag="E")
                        nc.scalar.activation(
                            E[:, :, w0:512],
                            sp[:].rearrange("p (two q) -> p two q", two=2)[:, :, w0:512],
                            EXP)
                        if diag:
                            # zero the strictly-masked staircase in the 128-wide
                            # diagonal block; columns beyond it are fully valid
                            nc.vector.tensor_mul(
                                E[:, :, w0:w0 + 128],
                                E[:, :, w0:w0 + 128],
                                strip_sb)
                        E_tiles[kt] = (E, w0)

                    # software pipeline: scores+exp run 2 k-tiles ahead of E@V,
                    # and ctx matmuls are grouped head-major per kt-pair so
                    # consecutive accumulates hit the same PSUM region
                    emit_scores_exp(0)
                    if fillers:
                        fillers.pop(0)()
                    emit_scores_exp(1)
                    if fillers:
                        fillers.pop(0)()
                    for kt0 in range(0, nkt, 2):
                        for k in (kt0 + 2, kt0 + 3):
                            if k < nkt:
                                emit_scores_exp(k)
                        for head in range(2):
                            for k in (kt0, kt0 + 1):
                                E, w0 = E_tiles[k]
                                nc.tensor.matmul(
                                    ctx2[:, head * 512 + w0:head * 512 + 512],
                                    lhsT=v2[:, pair, k, head * 65:head * 65 + 65],
                                    rhs=E[:, head, w0:512],
                                    start=(k == 0), stop=(k == nkt - 1),
                                )
                        E_tiles[kt0] = E_tiles[kt0 + 1] = None
                        # fill the PE's exp-wait slack with next-chunk qkv /
                        # previous-chunk outproj work
                        if fillers:
                            fillers.pop(0)()
                    # normalization: rows 64 hold the softmax denominators.
                    # copy to SBUF (recip reads partition 0 only), reciprocal,
                    # then a 64-partition gpsimd broadcast (both multiplies
                    # read base-partition-0 slices) — no DRAM round-trip.
                    r_sb = norm.tile([1, 1024], f32, tag="r_sb", name="r_sb")
                    nc.vector.tensor_copy(r_sb[0:1, :], ctx2[64:65, :])
                    rr = norm.tile([1, 1024], f32, tag="rr", name="rr")
                    nc.vector.reciprocal_approx_fast(out=rr[:], in_=r_sb[:])
                    rbc = norm.tile([64, 1024], f32, tag="rbc", name="rbc")
                    nc.gpsimd.partition_broadcast(rbc[:, :], rr[0:1, :])
                    nc.vector.tensor_mul(ctxT[qc][0:64, pair, :].bitcast(f32r), ctx2[0:64, 0:512], rbc[0:64, 0:512])
                    nc.vector.tensor_mul(ctxT[qc][64:128, pair, :].bitcast(f32r), ctx2[0:64, 512:1024], rbc[0:64, 512:1024])
                return fillers

            if mode == "causal":
                # attention(qc) only needs k-chunks <= qc. The next chunk's
                # QKV and the previous chunk's out-projection run as PE
                # fillers inside the attention loop.
                carry = emit_qkv(0, qk_only=True)
                for sc in range(SC):
                    fillers = list(carry)
                    carry = []
                    if sc + 1 < SC:
                        dma, qf = make_qkv_fillers(sc + 1)
                        dma()
                        fillers += qf
                    if sc > 0:
                        fillers += make_outproj_fillers(sc - 1)
                    rest = emit_attention(sc, fillers)
                    for f in rest:
                        f()
            else:
                # full attention needs every k-chunk before any q-chunk
                for sc in range(SC):
                    emit_qkv(sc)
                for qc in range(SC):
                    emit_attention(qc)
                    if qc > 0:
                        emit_outproj(qc - 1)
            emit_outproj(SC - 1)

    nc.compile()
    return nc


def _get_program(mode):
    if mode not in _CACHE:
        _CACHE[mode] = _build(mode)
    return _CACHE[mode]


def _classify_mask(mask):
    """Return "causal", "full", or "generic"."""
    m = mask.reshape(B, S, S)
    tril = np.tril_indices(S)
    if np.all(m == 0.0):
        return "full"
    for b in range(B):
        mb = m[b]
        if not np.all(mb[tril] == 0.0):
            return "generic"
        if not np.all(mb[np.triu_indices(S, k=1)] < -240.0):
            return "generic"
    return "causal"


def _prepare_in_maps(hidden_states, w_qkv, w_out):
    from ml_dtypes import bfloat16

    # strip[i, d, j] = 1.0 iff j >= i — the 128-wide diagonal staircase
    base = (np.arange(128, dtype=np.int32)[None, :] >= np.arange(128, dtype=np.int32)[:, None]).astype(bfloat16)
    strip = np.ascontiguousarray(np.broadcast_to(base[:, None, :], (128, 2, 128)))

    hT = [np.ascontiguousarray(hidden_states[b].T) for b in range(B)]

    in_maps = []
    for c in range(N_CORES):
        b, g = divmod(c, 4)
        cols = []
        for part in (0, 1, 2):  # q, k, v column groups of w_qkv
            for pair in range(2):
                for h in (4 * g + 2 * pair, 4 * g + 2 * pair + 1):
                    cols.append(w_qkv[:, part * H + h * DH: part * H + (h + 1) * DH])
        # order above is [q(pair0 h0,h1), q(pair1 h2,h3), k..., v...] = jt 0..5
        wqkv_c = np.concatenate(cols, axis=1)
        # fold the 1/sqrt(DH) score scale into the q columns
        wqkv_c = np.ascontiguousarray(wqkv_c)
        wqkv_c[:, 0:256] *= 1.0 / np.sqrt(DH)
        wo_c = np.ascontiguousarray(w_out[g * F_CORE:(g + 1) * F_CORE, :])
        in_maps.append({"hT": hT[b], "wqkv": wqkv_c, "wo": wo_c, "strip": strip,
                        "ident": np.eye(128, dtype=np.float32),
                        "ones": np.ones((128, 64), dtype=bfloat16)})
    return in_maps


def _run(inputs, trace=False):
    from concourse.bass_utils import run_bass_kernel_spmd

    hidden_states = np.asarray(inputs["hidden_states"], dtype=np.float32)
    mask = np.asarray(inputs["attention_mask"], dtype=np.float32)
    w_qkv = np.asarray(inputs["w_qkv"], dtype=np.float32)
    w_out = np.asarray(inputs["w_out"], dtype=np.float32)

    mode = _classify_mask(mask)
    if mode == "generic":
        return _numpy_reference(hidden_states, mask, w_qkv, w_out), None

    nc = _get_program(mode)
    in_maps = _prepare_in_maps(hidden_states, w_qkv, w_out)
    res = run_bass_kernel_spmd(nc, in_maps, list(range(N_CORES)), trace=trace)
    out = np.zeros((B, S, H), dtype=np.float32)
    for c in range(N_CORES):
        out[c // 4] += res.results[c]["out"]
    return out, res


def kernel(**inputs):
    out, _ = _run(inputs, trace=False)
    return out


def kernel_traced(**inputs):
    """Like kernel() but with NTFF profiling; returns (out, BassKernelResults)."""
    return _run(inputs, trace=True)


def _numpy_reference(hidden_states, mask, w_qkv, w_out):
    """Exact fallback for unrecognized masks (slow, chunked numpy)."""
    out = np.zeros((B, S, H), dtype=np.float32)
    m = mask.reshape(B, 1, S, S)
    for b in range(B):
        qkv = hidden_states[b] @ w_qkv  # [S, 3H]
        q = qkv[:, 0:H].reshape(S, NH, DH)
        k = qkv[:, H:2 * H].reshape(S, NH, DH)
        v = qkv[:, 2 * H:].reshape(S, NH, DH)
        ctx = np.zeros((S, NH, DH), dtype=np.float32)
        for h in range(NH):
            s = (q[:, h] @ k[:, h].T) / np.sqrt(DH) + m[b, 0]
            s = s - s.max(axis=-1, keepdims=True)
            e = np.exp(s)
            p = e / e.sum(axis=-1, keepdims=True)
            ctx[:, h] = p @ v[:, h]
        out[b] = ctx.reshape(S, H) @ w_out
    return out


# revision 55
# speedup vs baseline: 1.2919x; 1.1026x over previous
"""Multi-head attention Trainium2 kernel (8 NeuronCores, tensor+data parallel).

Problem: B=2, S=2048, H=1024, NH=16 heads, DH=64, causal additive mask.
  qkv = hs @ w_qkv ; per-head scaled-dot-product attention ; out = ctx @ w_out

Sharding: core c owns batch b=c//4 and 4 heads g=(c%4)*4..+4.  Each core
computes QKV^T for its head slice, attention in transposed-score layout
(softmax along the PSUM partition axis, sums via a ones-row augmented V),
and a partial out-projection over its 256 head features; the host sums the
4 partials per batch.

All matmuls run as float32r (FP22 multiply, FP32 accumulate) which streams
at 1 cycle/row on the PE like bf16 but with 13 mantissa bits.

Schedule notes (v2):
- softmax normalization broadcasts 1/denom across partitions with a gpsimd
  partition_broadcast instead of a DRAM DMA round-trip (the round-trip
  blocked the in-order vector queue ~10us per (qc, pair)).
- attention kt loop is software-pipelined: scores+exp are emitted two k-tiles
  ahead of the E@V matmuls so the PE never waits on the scalar-engine exp.
- input DMAs are spread across 4 engine queues so the first qkv matmul isn't
  stuck behind 5MB of consts on one queue.
- the causal staircase multiply only covers the 128-wide diagonal block of
  each score window (columns beyond it are fully below the diagonal).
- out-proj PSUM->SBUF staging copies run on gpsimd to unload the vector queue.
"""

import sys

sys.path.insert(0, "/opt/trn_rl_repo")

import numpy as np

B, S, H, NH = 2, 2048, 1024, 16
DH = H // NH  # 64
N_CORES = 8
HEADS_PER_CORE = 4  # 2 pairs
SC = S // 512  # 4 q/s chunks of 512
KT = S // 128  # 16 k tiles of 128
F_CORE = HEADS_PER_CORE * DH  # 256 out-proj features per core

_CACHE = {}


def _build(mode):
    """Build + schedule the Bass program for `mode` in {"causal", "full"}."""
    import concourse.bass as bass
    import concourse.mybir as mybir
    from concourse import bacc
    from concourse.tile import TileContext

    f32 = mybir.dt.float32
    f32r = mybir.dt.float32r
    bf16 = mybir.dt.bfloat16
    fp16 = mybir.dt.float16
    EXP = mybir.ActivationFunctionType.Exp

    nc = bacc.Bacc("TRN2", target_bir_lowering=False, debug=False,
                   num_devices=N_CORES)

    hT = nc.dram_tensor("hT", [H, S], f32, kind="ExternalInput").ap()
    wqkv = nc.dram_tensor("wqkv", [H, 6 * 128], f32, kind="ExternalInput").ap()
    wo = nc.dram_tensor("wo", [F_CORE, H], f32, kind="ExternalInput").ap()
    strip = nc.dram_tensor("strip", [128, 2, 128], bf16, kind="ExternalInput").ap()
    ident_d = nc.dram_tensor("ident", [128, 128], f32, kind="ExternalInput").ap()
    ones_d = nc.dram_tensor("ones", [128, 64], bf16, kind="ExternalInput").ap()
    out = nc.dram_tensor("out", [S, H], f32, kind="ExternalOutput").ap()

    def n_kt(qc):  # k-tiles needed for q chunk qc
        return 4 * qc + 4 if mode == "causal" else KT

    with TileContext(nc) as tc:
        with (
            tc.tile_pool(name="consts", bufs=1) as consts,
            tc.tile_pool(name="persist", bufs=1) as persist,
            tc.tile_pool(name="stream", bufs=2) as stream,
            tc.tile_pool(name="epool", bufs=6) as epool,
            tc.tile_pool(name="norm", bufs=3) as norm,
            tc.tile_pool(name="outp", bufs=2) as outp,
            tc.tile_pool(name="psA", bufs=3, space="PSUM") as psA,
            tc.tile_pool(name="psB", bufs=1, space="PSUM") as psB,
        ):
            # ---- tiles ----------------------------------------------------
            wqkv_sb = consts.tile([128, 8, 6 * 128], f32, tag="wqkv")
            wo_sb = consts.tile([128, 2, H], f32, tag="wo")
            ident = consts.tile([128, 128], f32, tag="ident")
            strip_sb = None
            if mode == "causal":
                # staircase for the 128-wide diagonal block only
                strip_sb = consts.tile([128, 2, 128], bf16, tag="strip", name="strip_sb")
            # q,k blocks per s-chunk in fp16 (halves score-matmul weight loads
            # and SBUF reads; 10-bit mantissa keeps probs within ~0.6%):
            # [p, jt, 512]; jt: 0,1=q pairs, 2,3=k pairs
            qkT = [persist.tile([128, 4, 512], fp16, name=f"qkT{sc}", tag=f"qkT{sc}") for sc in range(SC)]
            # v blocks stay f32 for the PE transposes: [p, pair, 512]
            vT = [persist.tile([128, 2, 512], f32, name=f"vT{sc}", tag=f"vT{sc}") for sc in range(SC)]
            # ctx^T blocks per q-chunk: [p(f within pair), pair, 512]
            ctxT = [persist.tile([128, 2, 512], f32, name=f"ctxT{qc}", tag=f"ctxT{qc}") for qc in range(SC)]
            # v in normal layout per pair: cols 0:64=headA, 64=ones, 65:129=headB, 129=ones
            v2 = persist.tile([128, 2, KT, 130], bf16, tag="v2")

            def emit_const_loads():
                # gpsimd queue: wqkv in 2-ht row groups, q+k columns first
                # (first needed by the jt loop and attention scores), v after
                wqkv_r = wqkv.rearrange("(ht p) j -> p ht j", p=128)
                for hg in range(4):
                    nc.gpsimd.dma_start(
                        out=wqkv_sb[:, 2 * hg:2 * hg + 2, 0:512].bitcast(f32r),
                        in_=wqkv_r[:, 2 * hg:2 * hg + 2, 0:512].bitcast(f32r))
                for hg in range(4):
                    nc.gpsimd.dma_start(
                        out=wqkv_sb[:, 2 * hg:2 * hg + 2, 512:768].bitcast(f32r),
                        in_=wqkv_r[:, 2 * hg:2 * hg + 2, 512:768].bitcast(f32r))
                # scalar queue: small consts needed early, then wo (needed last)
                nc.scalar.dma_start(out=ident[:].bitcast(f32r), in_=ident_d.bitcast(f32r))
                if strip_sb is not None:
                    nc.scalar.dma_start(out=strip_sb, in_=strip)
                nc.scalar.dma_start(
                    out=bass.AP(tensor=v2.tensor, offset=v2[:].offset + 64,
                                ap=[[v2[:].ap[0][0], 128], [65, 2 * KT * 2], [1, 1]]),
                    in_=bass.AP(tensor=ones_d.tensor, offset=0,
                                ap=[[64, 128], [0, 2 * KT * 2], [1, 1]]),
                )
                nc.scalar.dma_start(out=wo_sb.bitcast(f32r),
                                    in_=wo.rearrange("(ft p) e -> p ft e", p=128).bitcast(f32r))

            def emit_outproj(blk):
                """Final-block out-projection: staging copies alternate
                between scalar and vector, and each row-block's write is
                split across two DMA queues to halve the tail drain."""
                for i in range(4):
                    st = blk * 4 + i
                    po = psA.tile([128, 1024], f32, tag="psA", name="po")
                    for ec in range(2):
                        for ft in range(2):
                            nc.tensor.matmul(
                                po[:, ec * 512:(ec + 1) * 512],
                                lhsT=ctxT[blk][:, ft, i * 128:(i + 1) * 128].bitcast(f32r),
                                rhs=wo_sb[:, ft, ec * 512:(ec + 1) * 512].bitcast(f32r),
                                start=(ft == 0), stop=(ft == 1),
                            )
                    o_sb = outp.tile([128, 1024], f32, tag="o_sb", name="o_sb")
                    if i % 2 == 0:
                        nc.scalar.copy(o_sb, po)
                    else:
                        nc.vector.tensor_copy(o_sb, po)
                    nc.sync.dma_start(out=out[st * 128:(st + 1) * 128, :], in_=o_sb)

            def emit_qkv(sc, qk_only=False):
                # ==== QKV^T for s-chunk sc =================================
                hT_t = stream.tile([128, 8, 512], f32, tag="hT", name="hT_t")
                hT_r = hT.rearrange("(ht p) s -> p ht s", p=128)[:, :, sc * 512:(sc + 1) * 512]
                if sc == 0:
                    # per-ht splits across two queues so the first qkv matmul
                    # only waits for the first 256KB, not the full 2MB
                    for ht in range(4):
                        nc.sync.dma_start(out=hT_t[:, ht, :].bitcast(f32r), in_=hT_r[:, ht, :].bitcast(f32r))
                        nc.scalar.dma_start(out=hT_t[:, 4 + ht, :].bitcast(f32r), in_=hT_r[:, 4 + ht, :].bitcast(f32r))
                    emit_const_loads()
                else:
                    nc.sync.dma_start(out=hT_t.bitcast(f32r), in_=hT_r.bitcast(f32r))
                def emit_jt(jt):
                    ps = psA.tile([128, 1024], f32, tag="psA", name="ps")
                    acc = ps[:, 0:512]
                    for ht in range(8):
                        nc.tensor.matmul(
                            acc,
                            lhsT=wqkv_sb[:, ht, jt * 128:(jt + 1) * 128].bitcast(f32r),
                            rhs=hT_t[:, ht, :].bitcast(f32r),
                            start=(ht == 0),
                            stop=(ht == 7),
                        )
                    if jt < 4:
                        nc.vector.tensor_copy(qkT[sc][:, jt, :], acc)
                    else:
                        nc.vector.tensor_copy(vT[sc][:, jt - 4, :].bitcast(f32r), acc)

                # q,k first (scores), then each v block followed directly by
                # its transposes so v2 is ready when the ctx matmuls start
                for jt in range(4):
                    emit_jt(jt)
                if qk_only:
                    # v projections + transposes ride as attention(0) fillers
                    from functools import partial
                    return [partial(emit_jt, 4), partial(emit_transposes, sc, 0),
                            partial(emit_jt, 5), partial(emit_transposes, sc, 1)]
                emit_jt(4)
                emit_transposes(sc, 0)
                emit_jt(5)
                emit_transposes(sc, 1)
                return []

            def emit_transposes(sc, pair):
                # ==== v transposes for the 4 new k-tiles of one pair ========
                for i in range(4):
                    kt = sc * 4 + i
                    pst = psA.tile([128, 1024], f32, tag="psA", name="pst")
                    nc.tensor.transpose(
                        pst[:, 0:128].bitcast(f32r),
                        in_=vT[sc][:, pair, i * 128:(i + 1) * 128].bitcast(f32r),
                        identity=ident[:].bitcast(f32r),
                    )
                    # one copy per transpose (f32 PSUM -> bf16 SBUF):
                    # headA -> cols 0:64, headB -> 65:129
                    dst = v2[:, pair, kt, :]
                    nc.vector.tensor_copy(
                        bass.AP(tensor=v2.tensor, offset=dst.offset,
                                ap=[dst.ap[0], [65, 2], [1, 64]]),
                        pst[:, 0:128].rearrange("p (h d) -> p h d", h=2))

            def make_qkv_fillers(sc):
                """DMA starter + PE-filler closures for chunk sc's QKV.

                Each filler is ~1us of PE work (half a jt accumulation or a
                pair of V transposes) inserted into the exp-limited attention
                loop of the previous chunk, where the PE would otherwise idle.
                """
                st = {}

                def dma():
                    hT_t = stream.tile([128, 8, 512], f32, tag="hT", name="hT_t")
                    st["h"] = hT_t
                    hT_r = hT.rearrange("(ht p) s -> p ht s", p=128)[:, :, sc * 512:(sc + 1) * 512]
                    nc.sync.dma_start(out=hT_t.bitcast(f32r), in_=hT_r.bitcast(f32r))

                def jt_half(jt, half):
                    if half == 0:
                        st[jt] = psA.tile([128, 1024], f32, tag="psA", name="ps")
                    acc = st[jt][:, 0:512]
                    for ht in range(4 * half, 4 * half + 4):
                        nc.tensor.matmul(
                            acc,
                            lhsT=wqkv_sb[:, ht, jt * 128:(jt + 1) * 128].bitcast(f32r),
                            rhs=st["h"][:, ht, :].bitcast(f32r),
                            start=(ht == 0),
                            stop=(ht == 7),
                        )
                    if half == 1:
                        if jt < 4:
                            nc.vector.tensor_copy(qkT[sc][:, jt, :], acc)
                        else:
                            nc.vector.tensor_copy(vT[sc][:, jt - 4, :].bitcast(f32r), acc)
                        del st[jt]

                from functools import partial
                fillers = []
                for jt in (0, 1, 2, 3):
                    fillers.append(partial(jt_half, jt, 0))
                    fillers.append(partial(jt_half, jt, 1))
                fillers.append(partial(jt_half, 4, 0))
                fillers.append(partial(jt_half, 4, 1))
                fillers.append(partial(emit_transposes, sc, 0))
                fillers.append(partial(jt_half, 5, 0))
                fillers.append(partial(jt_half, 5, 1))
                fillers.append(partial(emit_transposes, sc, 1))
                return dma, fillers

            def make_outproj_fillers(blk):
                """Per-s-tile out-projection fillers (vector staging copy)."""
                def po_tile(i):
                    st_ = blk * 4 + i
                    po = psA.tile([128, 1024], f32, tag="psA", name="po")
                    for ec in range(2):
                        for ft in range(2):
                            nc.tensor.matmul(
                                po[:, ec * 512:(ec + 1) * 512],
                                lhsT=ctxT[blk][:, ft, i * 128:(i + 1) * 128].bitcast(f32r),
                                rhs=wo_sb[:, ft, ec * 512:(ec + 1) * 512].bitcast(f32r),
                                start=(ft == 0), stop=(ft == 1),
                            )
                    o_sb = outp.tile([128, 1024], f32, tag="o_sb", name="o_sb")
                    nc.vector.tensor_copy(o_sb, po)
                    nc.sync.dma_start(out=out[st_ * 128:(st_ + 1) * 128, :], in_=o_sb)

                from functools import partial
                return [partial(po_tile, i) for i in range(4)]

            def emit_attention(qc, fillers=(), lazy=()):
                # eager fillers (next chunk's qkv) pop at every slot; lazy
                # fillers (previous chunk's outproj) are held back so they
                # land near the end — including right after the final
                # normalization, where the PE would otherwise wait ~5us.
                fillers = list(fillers)
                lazy = list(lazy)
                nkt = n_kt(qc)
                slots_left = [2 * (2 + nkt // 2)]

                def pop_filler():
                    slots_left[0] -= 1
                    if fillers:
                        fillers.pop(0)()
                    elif lazy and slots_left[0] < len(lazy):
                        lazy.pop(0)()
                for pair in range(2):
                    # headA ctx in cols 0:512, headB in 512:1024 of one tile
                    ctx2 = psB.tile([65, 1024], f32, tag="ctx2", name="ctx2")
                    E_tiles = [None] * nkt

                    def emit_scores_exp(kt):
                        # diagonal tiles only need columns j >= 128*t
                        diag = mode == "causal" and kt >= 4 * qc
                        w0 = 128 * (kt - 4 * qc) if diag else 0
                        sp = psA.tile([128, 1024], f32, tag="psA")
                        kblk, ki = qkT[kt // 4], (kt % 4) * 128
                        # transposed scores, 2 heads row-packed on the PE
                        nc.tensor.matmul(
                            sp[:, w0:512],
                            lhsT=kblk[0:64, 2 + pair, ki:ki + 128],
                            rhs=qkT[qc][0:64, 0 + pair, w0:512],
                            start=True, stop=True,
                        )
                        nc.tensor.matmul(
                            sp[:, 512 + w0:1024],
                            lhsT=kblk[64:128, 2 + pair, ki:ki + 128],
                            rhs=qkT[qc][64:128, 0 + pair, w0:512],
                            start=True, stop=True,
                        )
                        E = epool.tile([128, 2, 512], bf16, tag="E")
                        nc.scalar.activation(
                            E[:, :, w0:512],
                            sp[:].rearrange("p (two q) -> p two q", two=2)[:, :, w0:512],
                            EXP)
                        if diag:
                            # zero the strictly-masked staircase in the 128-wide
                            # diagonal block; columns beyond it are fully valid
                            nc.vector.tensor_mul(
                                E[:, :, w0:w0 + 128],
                                E[:, :, w0:w0 + 128],
                                strip_sb)
                        E_tiles[kt] = (E, w0)

                    # software pipeline: scores+exp run 2 k-tiles ahead of E@V,
                    # and ctx matmuls are grouped head-major per kt-pair so
                    # consecutive accumulates hit the same PSUM region
                    emit_scores_exp(0)
                    pop_filler()
                    emit_scores_exp(1)
                    pop_filler()
                    for kt0 in range(0, nkt, 2):
                        for k in (kt0 + 2, kt0 + 3):
                            if k < nkt:
                                emit_scores_exp(k)
                        for head in range(2):
                            for k in (kt0, kt0 + 1):
                                E, w0 = E_tiles[k]
                                nc.tensor.matmul(
                                    ctx2[:, head * 512 + w0:head * 512 + 512],
                                    lhsT=v2[:, pair, k, head * 65:head * 65 + 65],
                                    rhs=E[:, head, w0:512],
                                    start=(k == 0), stop=(k == nkt - 1),
                                )
                        E_tiles[kt0] = E_tiles[kt0 + 1] = None
                        # fill the PE's exp-wait slack with next-chunk qkv /
                        # previous-chunk outproj work
                        pop_filler()
                    # normalization: rows 64 hold the softmax denominators.
                    # copy to SBUF (recip reads partition 0 only), reciprocal,
                    # then a 64-partition gpsimd broadcast (both multiplies
                    # read base-partition-0 slices) — no DRAM round-trip.
                    r_sb = norm.tile([1, 1024], f32, tag="r_sb", name="r_sb")
                    nc.vector.tensor_copy(r_sb[0:1, :], ctx2[64:65, :])
                    rr = norm.tile([1, 1024], f32, tag="rr", name="rr")
                    nc.vector.reciprocal_approx_fast(out=rr[:], in_=r_sb[:])
                    rbc = norm.tile([64, 1024], f32, tag="rbc", name="rbc")
                    nc.gpsimd.partition_broadcast(rbc[:, :], rr[0:1, :])
                    nc.vector.tensor_mul(ctxT[qc][0:64, pair, :].bitcast(f32r), ctx2[0:64, 0:512], rbc[0:64, 0:512])
                    nc.vector.tensor_mul(ctxT[qc][64:128, pair, :].bitcast(f32r), ctx2[0:64, 512:1024], rbc[0:64, 512:1024])
                return fillers + lazy

            if mode == "causal":
                # attention(qc) only needs k-chunks <= qc. The next chunk's
                # QKV and the previous chunk's out-projection run as PE
                # fillers inside the attention loop.
                carry = emit_qkv(0, qk_only=True)
                for sc in range(SC):
                    fillers = list(carry)
                    carry = []
                    if sc + 1 < SC:
                        dma, qf = make_qkv_fillers(sc + 1)
                        dma()
                        fillers += qf
                    lazy = make_outproj_fillers(sc - 1) if sc > 0 else []
                    rest = emit_attention(sc, fillers, lazy)
                    for f in rest:
                        f()
            else:
                # full attention needs every k-chunk before any q-chunk
                for sc in range(SC):
                    emit_qkv(sc)
                for qc in range(SC):
                    emit_attention(qc)
                    if qc > 0:
                        emit_outproj(qc - 1)
            emit_outproj(SC - 1)

    nc.compile()
    return nc


def _get_program(mode):
    if mode not in _CACHE:
        _CACHE[mode] = _build(mode)
    return _CACHE[mode]


def _classify_mask(mask):
    """Return "causal", "full", or "generic"."""
    m = mask.reshape(B, S, S)
    tril = np.tril_indices(S)
    if np.all(m == 0.0):
        return "full"
    for b in range(B):
        mb = m[b]
        if not np.all(mb[tril] == 0.0):
            return "generic"
        if not np.all(mb[np.triu_indices(S, k=1)] < -240.0):
            return "generic"
    return "causal"


def _prepare_in_maps(hidden_states, w_qkv, w_out):
    from ml_dtypes import bfloat16

    # strip[i, d, j] = 1.0 iff j >= i — the 128-wide diagonal staircase
    base = (np.arange(128, dtype=np.int32)[None, :] >= np.arange(128, dtype=np.int32)[:, None]).astype(bfloat16)
    strip = np.ascontiguousarray(np.broadcast_to(base[:, None, :], (128, 2, 128)))

    hT = [np.ascontiguousarray(hidden_states[b].T.astype(np.float16)) for b in range(B)]

    in_maps = []
    for c in range(N_CORES):
        b, g = divmod(c, 4)
        cols = []
        for part in (0, 1, 2):  # q, k, v column groups of w_qkv
            for pair in range(2):
                for h in (4 * g + 2 * pair, 4 * g + 2 * pair + 1):
                    cols.append(w_qkv[:, part * H + h * DH: part * H + (h + 1) * DH])
        # order above is [q(pair0 h0,h1), q(pair1 h2,h3), k..., v...] = jt 0..5
        wqkv_c = np.concatenate(cols, axis=1)
        # fold the 1/sqrt(DH) score scale into the q columns
        wqkv_c = np.ascontiguousarray(wqkv_c)
        wqkv_c[:, 0:256] *= 1.0 / np.sqrt(DH)
        wqkv_c = wqkv_c.astype(np.float16)
        wo_c = np.ascontiguousarray(w_out[g * F_CORE:(g + 1) * F_CORE, :])
        in_maps.append({"hT": hT[b], "wqkv": wqkv_c, "wo": wo_c, "strip": strip,
                        "ident": np.eye(128, dtype=np.float32),
                        "ones": np.ones((128, 64), dtype=bfloat16)})
    return in_maps


def _run(inputs, trace=False):
    from concourse.bass_utils import run_bass_kernel_spmd

    hidden_states = np.asarray(inputs["hidden_states"], dtype=np.float32)
    mask = np.asarray(inputs["attention_mask"], dtype=np.float32)
    w_qkv = np.asarray(inputs["w_qkv"], dtype=np.float32)
    w_out = np.asarray(inputs["w_out"], dtype=np.float32)

    mode = _classify_mask(mask)
    if mode == "generic":
        return _numpy_reference(hidden_states, mask, w_qkv, w_out), None

    nc = _get_program(mode)
    in_maps = _prepare_in_maps(hidden_states, w_qkv, w_out)
    res = run_bass_kernel_spmd(nc, in_maps, list(range(N_CORES)), trace=trace)
    out = np.zeros((B, S, H), dtype=np.float32)
    for c in range(N_CORES):
        out[c // 4] += res.results[c]["out"]
    return out, res


def kernel(**inputs):
    out, _ = _run(inputs, trace=False)
    return out


def kernel_traced(**inputs):
    """Like kernel() but with NTFF profiling; returns (out, BassKernelResults)."""
    return _run(inputs, trace=True)


def _numpy_reference(hidden_states, mask, w_qkv, w_out):
    """Exact fallback for unrecognized masks (slow, chunked numpy)."""
    out = np.zeros((B, S, H), dtype=np.float32)
    m = mask.reshape(B, 1, S, S)
    for b in range(B):
        qkv = hidden_states[b] @ w_qkv  # [S, 3H]
        q = qkv[:, 0:H].reshape(S, NH, DH)
        k = qkv[:, H:2 * H].reshape(S, NH, DH)
        v = qkv[:, 2 * H:].reshape(S, NH, DH)
        ctx = np.zeros((S, NH, DH), dtype=np.float32)
        for h in range(NH):
            s = (q[:, h] @ k[:, h].T) / np.sqrt(DH) + m[b, 0]
            s = s - s.max(axis=-1, keepdims=True)
            e = np.exp(s)
            p = e / e.sum(axis=-1, keepdims=True)
            ctx[:, h] = p @ v[:, h]
        out[b] = ctx.reshape(S, H) @ w_out
    return out
